# revision 9
# baseline (speedup 1.0000x reference)
"""Causal self-attention (B=4, S=2048, E=1024, H=16) on 8 TRN2 NeuronCores.

Sharding: core = (batch b, head-group g): b = core // 2, g = core % 2.
Each core handles one batch and 8 of the 16 heads (Megatron-style column
parallel QKV + row-parallel out-proj); the two half-projections per batch
are summed on the host.

All weights/activations are pre-transposed and cast to bf16 on the host so
the on-chip program is pure matmul + softmax:
  qT/kT [j=h*64+d, s] = WT.T @ xT        (heads on partitions)
  v     [s, j]        = xT.T @ WvT       (natural layout, + ones column)
  scT   [kj, qi]      = kT_h.T' @ qT_h   (K=64, 2 heads row-packed in PE)
  e = exp(0.125*scT) * causal_mask       (ScalarE from PSUM, DVE mask)
  pv    [65, qi]      = [v_h | 1].T @ e  (accumulated over kj; row 64 = rowsum)
  o     = pv[0:64] / pv[64]              (DVE fast recip + GpSimd part-bcast)
  outT  [e, s]        = WpT.T @ o_cat    (partial, bf16; host sums the groups)

Scheduling: the attention inner loop is ScalarE(exp)-paced, so all other
matmul streams (QKV projections, V, out-proj) are chopped into ~0.4-0.9us
"filler quanta" and woven into the attention emission at TILE granularity by
a virtual-clock greedy weaver: whenever the PE would otherwise wait for an
exp, it runs filler.  The kernel opens with an ek-major Q-projection
bootstrap that chases the x DMA chunk stream.  The softmax normalization
chain is split: the PSUM-freeing copies run at block end, the rest (rsum /
recip / broadcast / scale) is deferred into the next block's tile slots so
it never floods DVE at a block boundary.  Block order is qt = 0,1,3,2 and
the out-projection fillers are kept in reserve so the final attention block
still has PE work to hide its exps behind.  Output is bf16 (host sums f32).
"""

import sys

for _p in ("/opt/trn_rl_repo", "/root/.axon_site/_ro/trn_rl_repo"):
    if _p not in sys.path:
        sys.path.append(_p)

from contextlib import ExitStack

import numpy as np
import ml_dtypes

import concourse.bass as bass
import concourse.tile as tile
import concourse.mybir as mybir
from concourse import bacc
from concourse.bass_utils import run_bass_kernel_spmd

BF16 = mybir.dt.bfloat16
F32 = mybir.dt.float32
NP_BF16 = ml_dtypes.bfloat16

B, S, E, H = 4, 2048, 1024, 16
D = E // H            # 64
HL = H // 2           # 8 heads per core
JC = HL * D           # 512 local head-concat width
P = 128
NKT = S // P          # 16 key tiles
NQT = S // 512        # 4 query tiles of 512
EKT = E // P          # 8 contraction tiles for QKV projections
CT = JC // P          # 4 contraction tiles for the output projection
SCALE = 1.0 / np.sqrt(np.float32(D))  # 0.125

# virtual-clock estimates (ns)
CYC = 1.0 / 2.4
SEM = 100.0


def build_program(apply_key_mask: bool):
    nc = bacc.Bacc("TRN2", target_bir_lowering=False, debug=False, num_devices=8)

    xT = nc.dram_tensor("xT", [E, S], BF16, kind="ExternalInput").ap()
    wqT = nc.dram_tensor("wqT", [E, JC], BF16, kind="ExternalInput").ap()
    wkT = nc.dram_tensor("wkT", [E, JC], BF16, kind="ExternalInput").ap()
    wvT = nc.dram_tensor("wvT", [E, JC], BF16, kind="ExternalInput").ap()
    wpT = nc.dram_tensor("wpT", [JC, E], BF16, kind="ExternalInput").ap()
    if apply_key_mask:
        kmaskT = nc.dram_tensor("kmaskT", [P, NKT], F32, kind="ExternalInput").ap()
    outp = nc.dram_tensor("outp", [E, S], BF16, kind="ExternalOutput").ap()

    xT_r = xT.rearrange("(kt p) s -> p kt s", p=P)
    wq_r = wqT.rearrange("(kt p) j -> p kt j", p=P)
    wk_r = wkT.rearrange("(kt p) j -> p kt j", p=P)
    wv_r = wvT.rearrange("(kt p) j -> p kt j", p=P)

    with tile.TileContext(nc) as tc:
        with ExitStack() as ctx:
            per = ctx.enter_context(tc.tile_pool(name="per", bufs=1))
            sc_ps = ctx.enter_context(
                tc.tile_pool(name="sc_ps", bufs=2, space="PSUM")
            )
            pv_ps = ctx.enter_context(
                tc.tile_pool(name="pv_ps", bufs=2, space="PSUM")
            )
            fill_ps = ctx.enter_context(
                tc.tile_pool(name="fill_ps", bufs=2, space="PSUM")
            )
            esb = ctx.enter_context(tc.tile_pool(name="esb", bufs=3))
            nrm = ctx.enter_context(tc.tile_pool(name="nrm", bufs=6))
            posb = ctx.enter_context(tc.tile_pool(name="posb", bufs=3))

            # ---- input DMA: x chunk-wise on sync/gpsimd (chaseable);
            # weights interleaved wq/wk halves first on the scalar queue (7
            # configs total, all done before the first exp is needed).
            wq_sb = per.tile([P, EKT, JC], BF16, tag="wq")
            wk_sb = per.tile([P, EKT, JC], BF16, tag="wk")
            wv_sb = per.tile([P, EKT, JC], BF16, tag="wv")
            xT_sb = per.tile([P, EKT, S], BF16, tag="xT")
            for kt in range(EKT):
                (nc.sync if kt % 2 == 0 else nc.gpsimd).dma_start(
                    xT_sb[:, kt], xT_r[:, kt]
                )
            h0, h1 = slice(0, 4), slice(4, 8)
            nc.scalar.dma_start(wq_sb[:, h0], wq_r[:, h0])
            nc.scalar.dma_start(wk_sb[:, h0], wk_r[:, h0])
            nc.scalar.dma_start(wq_sb[:, h1], wq_r[:, h1])
            nc.scalar.dma_start(wk_sb[:, h1], wk_r[:, h1])
            nc.scalar.dma_start(wv_sb[:, h0], wv_r[:, h0])
            nc.scalar.dma_start(wv_sb[:, h1], wv_r[:, h1])
            wp_sb = per.tile([P, CT, E], BF16, tag="wp")
            nc.scalar.dma_start(wp_sb[:], wpT.rearrange("(ct p) e -> p ct e", p=P))
            if apply_key_mask:
                km_sb = per.tile([P, NKT], F32, tag="km")
                nc.sync.dma_start(km_sb[:], kmaskT[:])

            qT_sb = per.tile([P, CT, S], BF16, tag="qT")
            kT_sb = per.tile([P, CT, S], BF16, tag="kT")
            vaug_sb = per.tile([P, NKT, HL, D + 1], BF16, tag="vaug")
            o_sb = per.tile([P, CT, S], BF16, tag="o")

            nc.vector.memset(vaug_sb[:, :, :, D], 1.0)

            # on-chip causal mask constant for diagonal 128x128 subblocks:
            # cm[p, c] = 1 where c >= p else 0 (keys on partitions)
            cm_sb = per.tile([P, P], BF16, tag="cm")
            nc.gpsimd.memset(cm_sb[:], 1.0)
            nc.gpsimd.affine_select(
                out=cm_sb[:],
                in_=cm_sb[:],
                compare_op=mybir.AluOpType.is_ge,
                fill=0.0,
                base=0,
                pattern=[[1, P]],
                channel_multiplier=-1,
            )

            # dummy matmuls during the DMA-bound start: engage the HAM clock
            # gate before the first x chunks land so the bootstrap runs fast
            dum_a = per.tile([P, P], BF16, tag="dum_a")
            dum_b = per.tile([P, 256], BF16, tag="dum_b")
            nc.vector.memset(dum_a[:], 0.0)
            nc.vector.memset(dum_b[:], 0.0)
            dps = fill_ps.tile([P, 512], F32, tag="ps")
            for i in range(10):
                nc.tensor.matmul(
                    dps[:, 0:256], dum_a[:], dum_b[:],
                    start=(i == 0), stop=(i == 9),
                )

            # preload the exp table on ScalarE while DMAs stream in
            warm = nrm.tile([1, 16], F32, tag="warm")
            nc.vector.memset(warm[:], 0.0)
            warm2 = nrm.tile([1, 16], F32, tag="warm2")
            nc.scalar.activation(
                warm2[:], warm[:], mybir.ActivationFunctionType.Exp
            )

            # ================= emission helpers =================

            def emit_qk_unit(w_sb, dst, jt, st, ek_lo, ek_hi, ps_hold):
                if ek_lo == 0:
                    ps_hold["t"] = fill_ps.tile(
                        [P, 512], F32, tag="ps", name="fillps"
                    )
                ps = ps_hold["t"]
                for ek in range(ek_lo, ek_hi):
                    nc.tensor.matmul(
                        ps[:],
                        w_sb[:, ek, jt * P : (jt + 1) * P],
                        xT_sb[:, ek, st * 512 : (st + 1) * 512],
                        start=(ek == 0),
                        stop=(ek == EKT - 1),
                    )
                if ek_hi == EKT:
                    nc.vector.tensor_copy(
                        dst[:, jt, st * 512 : (st + 1) * 512], ps[:]
                    )

            def emit_v_unit(s128, ek_lo, ek_hi, ps_hold):
                if ek_lo == 0:
                    ps_hold["t"] = fill_ps.tile(
                        [P, 512], F32, tag="ps", name="fillps"
                    )
                ps = ps_hold["t"]
                for ek in range(ek_lo, ek_hi):
                    nc.tensor.matmul(
                        ps[:],
                        xT_sb[:, ek, s128 * P : (s128 + 1) * P],
                        wv_sb[:, ek, :],
                        start=(ek == 0),
                        stop=(ek == EKT - 1),
                    )
                if ek_hi == EKT:
                    nc.vector.tensor_copy(
                        vaug_sb[:, s128, :, 0:D],
                        ps[:].rearrange("p (h d) -> p h d", d=D),
                    )
                    if apply_key_mask:
                        nc.vector.tensor_scalar_mul(
                            vaug_sb[:, s128], vaug_sb[:, s128],
                            km_sb[:, s128 : s128 + 1],
                        )

            def emit_proj_unit(st, et, ct_lo, ct_hi, ps_hold, dma_eng=None):
                if ct_lo == 0:
                    ps_hold["t"] = fill_ps.tile(
                        [P, 512], F32, tag="ps", name="fillps"
                    )
                ps = ps_hold["t"]
                for ct in range(ct_lo, ct_hi):
                    nc.tensor.matmul(
                        ps[:],
                        wp_sb[:, ct, et * P : (et + 1) * P],
                        o_sb[:, ct, st * 512 : (st + 1) * 512],
                        start=(ct == 0),
                        stop=(ct == CT - 1),
                    )
                if ct_hi == CT:
                    po = posb.tile([P, 512], BF16, tag="po")
                    nc.vector.tensor_copy(po[:], ps[:])
                    (dma_eng or nc.sync).dma_start(
                        outp[et * P : (et + 1) * P, st * 512 : (st + 1) * 512],
                        po[:],
                    )

            # ================= the weaver =================

            class Weaver:
                def __init__(self):
                    self.items = []      # [gate, emit_fn, pe_ns, tag]
                    self.deferred = []   # closures (non-PE work, e.g. norm)
                    self.progress = 0
                    self.pe_t = 0.0
                    self.sc_t = 0.0

                def add(self, emit, ns, tag=None, gate=0, unit=None):
                    self.items.append([gate, emit, ns, tag, unit])

                def pe(self, ns):
                    self.pe_t += ns

                def drain_unit(self, unit):
                    # emit (in list order) only the quanta belonging to this
                    # unit, leaving everything else in reserve
                    picked = [it for it in self.items if it[4] == unit]
                    for it in picked:
                        self.items.remove(it)
                        it[1]()
                        self.pe_t += it[2]

                def has(self, unit):
                    return any(it[4] == unit for it in self.items)

                def fill(self, target):
                    while self.pe_t < target:
                        pick = None
                        for it in self.items:
                            if it[0] <= self.progress:
                                pick = it
                                break
                        if pick is None:
                            return
                        self.items.remove(pick)
                        pick[1]()
                        self.pe_t += pick[2]

                def push_deferred(self, closures):
                    self.deferred.extend(closures)

                def pop_deferred(self):
                    if self.deferred:
                        self.deferred.pop(0)()

                def flush_deferred(self):
                    while self.deferred:
                        self.deferred.pop(0)()

                def drain_all(self):
                    while self.items:
                        it = self.items.pop(0)
                        it[1]()
                        self.pe_t += it[2]

            W = Weaver()

            # ---- bootstrap: q-projection jt=0, all st, ek-major (chases the
            # x chunk stream); then kT jt=0 st=0 so attention can start.
            qA = sc_ps.tile([P, 1024], F32, tag="sc")
            qB = sc_ps.tile([P, 1024], F32, tag="sc")
            q_groups = [
                qA[:, 0:512], qA[:, 512:1024], qB[:, 0:512], qB[:, 512:1024]
            ]
            for ek in range(EKT):
                for st in range(NQT):
                    nc.tensor.matmul(
                        q_groups[st],
                        wq_sb[:, ek, 0:P],
                        xT_sb[:, ek, st * 512 : (st + 1) * 512],
                        start=(ek == 0),
                        stop=(ek == EKT - 1),
                        skip_group_check=True,
                    )
            W.pe(EKT * NQT * 512 * CYC)
            for st in range(NQT):
                nc.vector.tensor_copy(
                    qT_sb[:, 0, st * 512 : (st + 1) * 512], q_groups[st]
                )

            k0_ps = pv_ps.tile([P, 512], F32, tag="pv")
            for ek in range(EKT):
                nc.tensor.matmul(
                    k0_ps[:],
                    wk_sb[:, ek, 0:P],
                    xT_sb[:, ek, 0:512],
                    start=(ek == 0),
                    stop=(ek == EKT - 1),
                    skip_group_check=True,
                )
            W.pe(EKT * 512 * CYC)
            nc.vector.tensor_copy(kT_sb[:, 0, 0:512], k0_ps[:])

            # ---- filler inventory (order matters: the weaver consumes from
            # the front; projections sit at the back as tail reserve)
            def add_qk(jt, st, w_sb, dst, pre):
                hold = {}
                u = f"{pre}{jt}_{st}"
                W.add(lambda h=hold: emit_qk_unit(w_sb, dst, jt, st, 0, 4, h),
                      4 * 512 * CYC, unit=u)
                W.add(lambda h=hold: emit_qk_unit(w_sb, dst, jt, st, 4, 8, h),
                      4 * 512 * CYC, unit=u)

            def add_v(s128):
                hold = {}
                u = f"v{s128}"
                W.add(lambda h=hold: emit_v_unit(s128, 0, 4, h),
                      4 * 512 * CYC, unit=u)
                W.add(lambda h=hold: emit_v_unit(s128, 4, 8, h),
                      4 * 512 * CYC, unit=u)

            def add_proj(st, gate, engs=None):
                for et in range(8):
                    hold = {}
                    eng = engs[et % len(engs)] if engs else None
                    W.add(lambda e=et, h=hold: emit_proj_unit(st, e, 0, 2, h),
                          2 * 512 * CYC, gate=gate)
                    W.add(lambda e=et, h=hold, g=eng:
                          emit_proj_unit(st, e, 2, 4, h, dma_eng=g),
                          2 * 512 * CYC, gate=gate)

            for st in range(1, NQT):
                hold = {}
                W.add(lambda s=st, h=hold:
                      emit_qk_unit(wk_sb, kT_sb, 0, s, 0, 4, h),
                      4 * 512 * CYC, unit=f"k0_{st}")
                W.add(lambda s=st, h=hold:
                      emit_qk_unit(wk_sb, kT_sb, 0, s, 4, 8, h),
                      4 * 512 * CYC, unit=f"k0_{st}")
            # qk st0 before the v units: v depends on wv which lands last in
            # the weight DMA stream, so give the PE wq/wk-gated work first
            for a in range(1, 4):
                add_qk(a, 0, wq_sb, qT_sb, "q")
                add_qk(a, 0, wk_sb, kT_sb, "k")
            for s128 in range(4):
                add_v(s128)
            for s128 in range(4, 8):
                add_v(s128)
            for a in range(1, 4):
                add_qk(a, 1, wq_sb, qT_sb, "q")
                add_qk(a, 1, wk_sb, kT_sb, "k")
            for s128 in range(8, 12):
                add_v(s128)
            for a in range(1, 4):
                add_qk(a, 2, wq_sb, qT_sb, "q")
                add_qk(a, 2, wk_sb, kT_sb, "k")
            for s128 in range(12, 16):
                add_v(s128)
            for a in range(1, 4):
                add_qk(a, 3, wq_sb, qT_sb, "q")
                add_qk(a, 3, wk_sb, kT_sb, "k")
            # projections: consumed late; proj(2) is the structural tail and
            # proj(0) sits just before it as the PE's work during the final
            # normalization latency.
            add_proj(3, gate=12)
            add_proj(1, gate=8)
            add_proj(0, gate=4)
            add_proj(2, gate=16, engs=[nc.sync, nc.scalar, nc.gpsimd])

            # ---- attention blocks, tile-granular weave
            def emit_attn(qt, a):
                qs0 = qt * 512
                nkt = 4 * qt + 4
                pv = pv_ps.tile([P, 512], F32, tag="pv")
                pv2 = pv_ps.tile([P, 512], F32, tag="pv")

                def emit_sc(kt):
                    r = kt - 4 * qt
                    c0 = 128 * r if r > 0 else 0
                    ks = slice(kt * P, (kt + 1) * P)
                    qs = slice(qs0 + c0, qs0 + 512)
                    sc = sc_ps.tile([P, 1024], F32, tag="sc")
                    nc.tensor.matmul(
                        sc[:, c0:512],
                        kT_sb[0:D, a, ks],
                        qT_sb[0:D, a, qs],
                        start=True, stop=True,
                    )
                    nc.tensor.matmul(
                        sc[:, 512 + c0 : 1024],
                        kT_sb[D : 2 * D, a, ks],
                        qT_sb[D : 2 * D, a, qs],
                        start=True, stop=True,
                    )
                    W.pe(2 * (512 - c0) * CYC + 40)
                    return sc

                def emit_exp(sc, kt):
                    r = kt - 4 * qt
                    c0 = 128 * r if r > 0 else 0
                    e = esb.tile([P, 1024], BF16, tag="e")
                    e2 = e[:].rearrange("p (two c) -> p two c", two=2)
                    sc2 = sc[:].rearrange("p (two c) -> p two c", two=2)
                    nc.scalar.activation(
                        e2[:, :, c0:512], sc2[:, :, c0:512],
                        mybir.ActivationFunctionType.Exp,
                        scale=float(SCALE),
                    )
                    exp_ns = 2 * (512 - c0) * 0.8333 + 265
                    start = max(W.sc_t, W.pe_t + SEM)
                    W.sc_t = start + exp_ns
                    dep_end = W.sc_t
                    if r >= 0:
                        nc.vector.tensor_mul(
                            e2[:, :, c0 : c0 + 128],
                            e2[:, :, c0 : c0 + 128],
                            cm_sb[:, None, :].to_broadcast((P, 2, P)),
                        )
                        dep_end += 400 + SEM
                    return e, dep_end

                def emit_pv(e, kt, last):
                    r = kt - 4 * qt
                    c0 = 128 * r if r > 0 else 0
                    nc.tensor.matmul(
                        pv[0 : D + 1, c0:512],
                        vaug_sb[:, kt, 2 * a, :],
                        e[:, c0:512],
                        start=(kt == 0), stop=last,
                        skip_group_check=True,
                    )
                    nc.tensor.matmul(
                        pv2[0 : D + 1, c0:512],
                        vaug_sb[:, kt, 2 * a + 1, :],
                        e[:, 512 + c0 : 1024],
                        start=(kt == 0), stop=last,
                        skip_group_check=True,
                    )
                    W.pe(2 * (512 - c0) * CYC + 40)

                sc = emit_sc(0)
                for kt in range(nkt):
                    e, dep_end = emit_exp(sc, kt)
                    if kt + 1 < nkt:
                        sc = emit_sc(kt + 1)
                    if W.has(f"v{kt}"):
                        W.drain_unit(f"v{kt}")
                    W.fill(dep_end + 250)
                    if W.pe_t < dep_end + 150:
                        W.pe_t = dep_end + 150
                    emit_pv(e, kt, last=(kt == nkt - 1))
                    W.pop_deferred()

                # norm part 1 (immediate): copy PV rows out of PSUM so the
                # pv banks free for the next block's accumulation.
                qsl = slice(qs0, qs0 + 512)
                u1 = nrm.tile([D + 1, 512], F32, tag="unorm")
                nc.scalar.activation(
                    u1[:], pv[0 : D + 1, :],
                    mybir.ActivationFunctionType.Copy,
                )
                u2 = nrm.tile([D + 1, 512], F32, tag="unorm")
                nc.scalar.activation(
                    u2[:], pv2[0 : D + 1, :],
                    mybir.ActivationFunctionType.Copy,
                )

                # norm part 2 (deferred into the next block's tile slots):
                # rsum staging + recip + partition-broadcast + scale.
                state = {}

                def d_recip():
                    rs1 = nrm.tile([1, 512], F32, tag="rsum", name="rs1")
                    nc.vector.tensor_copy(rs1[:], u1[D : D + 1, :])
                    rc1 = nrm.tile([1, 512], F32, tag="rec", name="rc1")
                    nc.vector.reciprocal_approx_fast(rc1[:], rs1[:])
                    rs2 = nrm.tile([1, 512], F32, tag="rsum", name="rs2")
                    nc.vector.tensor_copy(rs2[:], u2[D : D + 1, :])
                    rc2 = nrm.tile([1, 512], F32, tag="rec", name="rc2")
                    nc.vector.reciprocal_approx_fast(rc2[:], rs2[:])
                    state["rc"] = (rc1, rc2)

                def d_bcast():
                    rc1, rc2 = state["rc"]
                    bc1 = nrm.tile([D, 512], F32, tag="bc", name="bc1")
                    nc.gpsimd.partition_broadcast(bc1[:], rc1[:])
                    bc2 = nrm.tile([D, 512], F32, tag="bc", name="bc2")
                    nc.gpsimd.partition_broadcast(bc2[:], rc2[:])
                    state["bc"] = (bc1, bc2)

                def d_scale():
                    bc1, bc2 = state["bc"]
                    nc.vector.tensor_mul(o_sb[0:D, a, qsl], u1[0:D, :], bc1[:])
                    tmp = nrm.tile([D, 512], BF16, tag="tmp")
                    nc.vector.tensor_mul(tmp[:], u2[0:D, :], bc2[:])
                    nc.gpsimd.dma_start(o_sb[D : 2 * D, a, qsl], tmp[:])

                def d_done():
                    W.progress += 1

                W.push_deferred([d_recip, d_bcast, d_scale, d_done])

            for qt in (0, 1, 3, 2):
                for a in range(CT):
                    if a >= 1:
                        if W.has(f"q{a}_{qt}"):
                            W.drain_unit(f"q{a}_{qt}")
                        for s in range(qt + 1):
                            if W.has(f"k{a}_{s}"):
                                W.drain_unit(f"k{a}_{s}")
                    else:
                        for s in range(1, qt + 1):
                            if W.has(f"k0_{s}"):
                                W.drain_unit(f"k0_{s}")
                    emit_attn(qt, a)

            W.flush_deferred()
            W.drain_all()

    nc.compile()
    return nc


def kernel(input, attention_mask, Wq, Wk, Wv, Wp, _profile=False):
    input = np.asarray(input, dtype=np.float32)
    attention_mask = np.asarray(attention_mask)
    Wq, Wk, Wv, Wp = (np.asarray(w, dtype=np.float32) for w in (Wq, Wk, Wv, Wp))

    mask_all = bool(attention_mask.all())
    nc = build_program(apply_key_mask=not mask_all)

    in_maps = []
    for core in range(8):
        b, g = core // 2, core % 2
        rows = slice(g * JC, (g + 1) * JC)
        m = {
            "xT": np.ascontiguousarray(input[b].T).astype(NP_BF16),
            "wqT": np.ascontiguousarray(Wq[rows].T).astype(NP_BF16),
            "wkT": np.ascontiguousarray(Wk[rows].T).astype(NP_BF16),
            "wvT": np.ascontiguousarray(Wv[rows].T).astype(NP_BF16),
            "wpT": np.ascontiguousarray(Wp[:, rows].T).astype(NP_BF16),
        }
        if not mask_all:
            km = attention_mask[b].astype(np.float32)  # [S]
            m["kmaskT"] = np.ascontiguousarray(km.reshape(NKT, P).T)
        in_maps.append(m)

    res = run_bass_kernel_spmd(
        nc, in_maps, core_ids=list(range(8)), trace=_profile
    )

    out = np.empty((B, S, E), dtype=np.float32)
    for b in range(B):
        acc = (res.results[2 * b]["outp"].astype(np.float32)
               + res.results[2 * b + 1]["outp"].astype(np.float32))
        out[b] = acc.T
    if _profile:
        return out, res
    return out
